# revision 13
# baseline (speedup 1.0000x reference)
"""Trainium2 Bass kernel for nn_BackwardStep_38749194944853.

Batched ADMM QP solve (OSQP-style), N=1024 independent QPs of dim nx=128 with
mi=128 inequality + me=32 doubled equality constraints, 100 fixed iterations.

Strategy (pure data-parallel over 8 cores, 128 QPs per core):
  Phase A (per element, TensorE-heavy):
    K = Q + (1+sigma) I + rho (Ai'Ai + 2 Ae'Ae)
    Kinv via Newton-Schulz (scalar init c*I, c = 2/(lam_lo+lam_hi); K >= 1.1 I
    by construction, lam_max <= ~6.8 measured, bound 7.5)
    M = Kinv At'  (At = [Ai; Ae], 160x128);  G = At M (160x160, symmetric)
    d = At (-Kinv qv) = H nqv;  s_vec = -Kinv qv
    Stored stationary tiles: T1 = -[G[0:128,:] | d_top], T2 = -[G[128:160,:] | d_bot]
    H = At Kinv (=M^T) spilled to DRAM for the final matvec.
  Phase B (98 iterations), state s_t = a_t - u in blocks [i(128); e2(32); e3(32)],
  laid out [m-partitions, element-columns]:
    B   = |rho s|           (ScalarE Abs with scale)
    p~  = [B_i ; B_e2-B_e3] (matvec input, per-element column)
    s' = C' + (0.5/rho) B + 0.5 s - G p~   (+G p~ for the e3 block)
    accumulated directly in PSUM: batched identity-stationary matmuls for the
    affine terms + per-element 4 matmuls with the stored -G tiles.
  Final: x = M (rho uC - p~_99) + s_vec via the spilled H as stationary.

Numerics validated vs fp64 replica of the reference: rel err ~8e-7 (fp32).
"""
import os
import numpy as np

import concourse.bass as bass
import concourse.bacc as bacc
import concourse.mybir as mybir
from concourse.tile import TileContext
from concourse.masks import make_identity
from concourse.bass_utils import run_bass_kernel_spmd

F32 = mybir.dt.float32
ALU = mybir.AluOpType
AFT = mybir.ActivationFunctionType

NCORES = 8
P = 128            # elements per core
NX = 128           # QP dimension
MI = 128           # inequality rows
ME = 32            # equality rows
MT = MI + ME       # 160 collapsed constraint dim
GW = MT + 1        # 161: per-element stationary tile width (G row | d entry)

RHO = 0.1
EPS_ = 1e-4
ACOEF = 1.0 + 1e-6          # alpha + sigma added to Q's diagonal
C0 = float(2.0 / (1.1 + 7.5))  # Newton-Schulz scalar init
NS_LOOP = 7                  # NS iterations after the fused first one (8 total)
N_ITER = 100                 # reference ADMM iterations
N_AUPD = N_ITER - 2          # 98 a-state updates (a_1 given, w from a_99)
HALF_PER_BODY = 2            # ADMM iterations per For_i body
N_BODY = N_AUPD // HALF_PER_BODY  # 49


def _col(t, n):
    """Column n of a 2D tile as an AP of shape [p, 1]."""
    return t[:, n:n + 1]


def _strided_cols(t, start, step, count, part=None):
    """AP selecting `count` columns of tile `t` starting at `start`, stride `step`."""
    base = t[:, 0:1] if part is None else t[part[0]:part[1], 0:1]
    return bass.AP(tensor=base.tensor, offset=base.offset + start,
                   ap=[base.ap[0], [step, count]])


def build(n_el=P, n_body=N_BODY, ns_loop=NS_LOOP, taps=False):
    nc = bacc.Bacc()

    x_d = nc.dram_tensor("x", [P, NX, 1], F32, kind="ExternalInput")
    Q_d = nc.dram_tensor("Q", [P, NX, NX], F32, kind="ExternalInput")
    q_d = nc.dram_tensor("q", [P, NX, 1], F32, kind="ExternalInput")
    Ai_d = nc.dram_tensor("A_ineq", [P, MI, NX], F32, kind="ExternalInput")
    bi_d = nc.dram_tensor("b_ineq", [P, MI, 1], F32, kind="ExternalInput")
    Ae_d = nc.dram_tensor("A_eq", [P, ME, NX], F32, kind="ExternalInput")
    be_d = nc.dram_tensor("b_eq", [P, ME, 1], F32, kind="ExternalInput")
    out_d = nc.dram_tensor("out", [P, NX, 1], F32, kind="ExternalOutput")
    hsp_d = nc.dram_tensor("hspill", [P, MT, NX], F32)  # internal DRAM
    if taps:
        dbg_d = nc.dram_tensor("dbg", [8, 128, 256], F32, kind="ExternalOutput")

    with TileContext(nc) as tc:
        with (
            tc.tile_pool(name="consts", bufs=1) as consts,
            tc.tile_pool(name="gpool", bufs=1) as gpool,
            tc.tile_pool(name="work", bufs=3) as work,
            tc.tile_pool(name="wks", bufs=2) as wks,
            tc.tile_pool(name="hre", bufs=8) as hre,
            tc.tile_pool(name="pspool", bufs=7, space="PSUM") as pspool,
            tc.tile_pool(name="sbpool", bufs=1, space="PSUM") as sbpool,
        ):
            # ---------------- constants ----------------
            ident = consts.tile([128, 128], F32)
            make_identity(nc, ident)
            negI = consts.tile([128, 128], F32)
            nc.vector.tensor_scalar_mul(negI, ident, -1.0)
            halfI = consts.tile([128, 128], F32)
            nc.vector.tensor_scalar_mul(halfI, ident, 0.5)
            hbrI = consts.tile([128, 128], F32)
            nc.vector.tensor_scalar_mul(hbrI, ident, 0.5 / RHO)
            twoI = consts.tile([128, 128], F32)
            nc.vector.tensor_scalar_mul(twoI, ident, 2.0)
            twoCid = consts.tile([128, 128], F32)
            nc.vector.tensor_scalar_mul(twoCid, ident, 2.0 * C0)
            cIdent = consts.tile([128, 128], F32)
            nc.vector.tensor_scalar_mul(cIdent, ident, ACOEF)

            # ---------------- persistent big tiles ----------------
            # T1_all: per element -[G[0:128, 0:160] | d_top]   [128, n_el*161]
            T1_all = gpool.tile([128, n_el * GW], F32)
            # G2_all: per element -[G[128:160, 0:160] | d_bot], packed 4 elements
            # per partition-group: element n lives at partitions 32*(n%4),
            # free slice (n//4)*161.
            G2_all = gpool.tile([128, (n_el // 4) * GW], F32)

            def t1(n):
                return T1_all[:, n * GW:(n + 1) * GW]

            def g2(n):
                a, g = n % 4, n // 4
                return G2_all[32 * a:32 * a + 32, g * GW:(g + 1) * GW]

            # batched constant tiles (m-layout: [m-part, element-cols])
            u_i = gpool.tile([128, n_el], F32)
            be_t = gpool.tile([32, n_el], F32)
            u_e2 = gpool.tile([32, n_el], F32)
            ruC_top = gpool.tile([128, n_el], F32)
            ruC_bot = gpool.tile([32, n_el], F32)
            nruC_top = gpool.tile([128, n_el], F32)
            nruC_bot = gpool.tile([128, n_el], F32)   # replicated x4
            nqv_all = gpool.tile([128, n_el], F32)
            Cp_i = gpool.tile([128, n_el], F32)
            Cp_e = gpool.tile([32, 2 * n_el], F32)    # [Cp_e2 | Cp_e3]
            S_all = gpool.tile([128, n_el], F32)
            # ADMM state (ping-pong a/b)
            s_i = [gpool.tile([128, n_el], F32, name=f"s_i{j}") for j in range(2)]
            s_e = [gpool.tile([32, 2 * n_el], F32, name=f"s_e{j}") for j in range(2)]
            B_i = [gpool.tile([128, n_el], F32, name=f"B_i{j}") for j in range(2)]
            B_e = [gpool.tile([32, 2 * n_el], F32, name=f"B_e{j}") for j in range(2)]
            pbot = [gpool.tile([128, n_el], F32, name=f"pbot{j}") for j in range(2)]
            he_sb = [gpool.tile([32, n_el], F32, name=f"he_sb{j}") for j in range(2)]
            f_top = gpool.tile([128, n_el], F32)
            f_bot = gpool.tile([32, n_el], F32)
            xo = gpool.tile([128, n_el], F32)
            xout = gpool.tile([n_el, 128], F32)

            Sbank = sbpool.tile([128, n_el], F32, tag="sbank")

            # ---------------- batched input prep ----------------
            x_el = wks.tile([P, NX], F32, tag="xel")
            q_el = wks.tile([P, NX], F32, tag="qel")
            nc.sync.dma_start(out=x_el, in_=x_d[:, :, 0])
            nc.sync.dma_start(out=q_el, in_=q_d[:, :, 0])
            nq_el = wks.tile([P, NX], F32, tag="nqel")
            nc.vector.tensor_tensor(nq_el, x_el, q_el, ALU.subtract)  # -(q - x)
            nqps = pspool.tile([128, P], F32, tag="ps")
            nc.tensor.transpose(nqps, nq_el, ident)
            nc.vector.tensor_copy(nqv_all, nqps[:, 0:n_el])

            bi_el = wks.tile([P, MI], F32, tag="biel")
            nc.sync.dma_start(out=bi_el, in_=bi_d[:, :, 0])
            bips = pspool.tile([128, P], F32, tag="ps")
            nc.tensor.transpose(bips, bi_el, ident)
            nc.vector.tensor_copy(u_i, bips[:, 0:n_el])

            be_el = wks.tile([P, ME], F32, tag="beel")
            nc.sync.dma_start(out=be_el, in_=be_d[:, :, 0])
            beps = pspool.tile([32, P], F32, tag="ps")
            nc.tensor.transpose(beps, be_el, ident)
            nc.vector.tensor_copy(be_t, beps[:, 0:n_el])

            nc.vector.tensor_scalar_add(u_e2, be_t, EPS_)
            nc.vector.tensor_scalar_mul(ruC_top, u_i, RHO)
            nc.vector.tensor_scalar(out=ruC_bot, in0=be_t, scalar1=2.0 * RHO,
                                    scalar2=RHO * EPS_, op0=ALU.mult, op1=ALU.add)
            nc.vector.tensor_scalar_mul(nruC_top, u_i, -RHO)
            nc.vector.tensor_scalar(out=nruC_bot[0:32, :], in0=be_t,
                                    scalar1=-2.0 * RHO, scalar2=-RHO * EPS_,
                                    op0=ALU.mult, op1=ALU.add)
            nc.vector.tensor_copy(nruC_bot[32:64, :], nruC_bot[0:32, :])
            nc.vector.tensor_copy(nruC_bot[64:128, :], nruC_bot[0:64, :])

            # ---------------- phase A: per-element factorization ----------------
            for n in range(n_el):
                Qt = work.tile([128, 128], F32, tag="Q")
                nc.sync.dma_start(out=Qt, in_=Q_d[n])
                Ait = work.tile([128, 128], F32, tag="Ai")
                nc.sync.dma_start(out=Ait, in_=Ai_d[n])
                Aet = work.tile([32, 128], F32, tag="Ae")
                nc.sync.dma_start(out=Aet, in_=Ae_d[n])

                at_ps = pspool.tile([128, MT], F32, tag="ps")
                nc.tensor.transpose(at_ps[:, 0:128], Ait, ident)
                nc.tensor.transpose(at_ps[:, 128:160], Aet, ident[0:32, 0:32])
                AT = work.tile([128, MT], F32, tag="AT")
                nc.vector.tensor_copy(AT, at_ps)

                AiS = work.tile([128, 128], F32, tag="AiS")
                nc.vector.tensor_scalar_mul(AiS, Ait, RHO)
                AeS = work.tile([32, 128], F32, tag="AeS")
                nc.scalar.activation(AeS, Aet, AFT.Copy, scale=2.0 * RHO)

                K_ps = pspool.tile([128, 128], F32, tag="ps")
                nc.tensor.matmul(K_ps, Ait, AiS, start=True, stop=False)
                nc.tensor.matmul(K_ps, Aet, AeS, start=False, stop=False)
                nc.tensor.matmul(K_ps, ident, Qt, start=False, stop=True)
                negK = work.tile([128, 128], F32, tag="negK")
                nc.vector.scalar_tensor_tensor(out=negK, in0=K_ps, scalar=-1.0,
                                               in1=cIdent, op0=ALU.mult,
                                               op1=ALU.subtract)
                X = work.tile([128, 128], F32, tag="X")
                nc.vector.scalar_tensor_tensor(out=X, in0=negK, scalar=C0 * C0,
                                               in1=twoCid, op0=ALU.mult,
                                               op1=ALU.add)
                for _ in range(ns_loop):
                    G1_ps = pspool.tile([128, 128], F32, tag="ps")
                    nc.tensor.matmul(G1_ps, negK, X, start=True, stop=True)
                    g1 = work.tile([128, 128], F32, tag="g1")
                    nc.scalar.activation(g1, G1_ps, AFT.Copy)
                    X2_ps = pspool.tile([128, 128], F32, tag="ps")
                    nc.tensor.matmul(X2_ps, X, g1, start=True, stop=False)
                    nc.tensor.matmul(X2_ps, twoI, X, start=False, stop=True)
                    Xn = work.tile([128, 128], F32, tag="X")
                    nc.vector.tensor_copy(Xn, X2_ps)
                    X = Xn

                # M = Kinv At' ; s_vec column into persistent Sbank
                Ms_ps = pspool.tile([128, MT], F32, tag="ps")
                nc.tensor.matmul(Ms_ps, X, AT, start=True, stop=True)
                nc.tensor.matmul(_col(Sbank, n), X, _col(nqv_all, n),
                                 start=True, stop=True, skip_group_check=True)
                Ms = work.tile([128, MT], F32, tag="Ms")
                nc.vector.tensor_copy(Ms, Ms_ps)

                # H = At Kinv  (two chunks into one psum bank)
                H_ps = pspool.tile([128, 256], F32, tag="ps")
                nc.tensor.matmul(H_ps[:, 0:128], AT[:, 0:128], X,
                                 start=True, stop=False, skip_group_check=True)
                nc.tensor.matmul(H_ps[0:32, 128:256], AT[:, 128:160], X,
                                 start=False, stop=True, skip_group_check=True)

                # G rows + d entries
                Gr1_ps = pspool.tile([128, GW], F32, tag="ps")
                nc.tensor.matmul(Gr1_ps[:, 0:MT], AT[:, 0:128], Ms,
                                 start=True, stop=False, skip_group_check=True)
                nc.tensor.matmul(Gr1_ps[:, MT:GW], Ms[:, 0:128], _col(nqv_all, n),
                                 start=True, stop=True, skip_group_check=True)
                Gr2_ps = pspool.tile([32, GW], F32, tag="ps")
                nc.tensor.matmul(Gr2_ps[:, 0:MT], AT[:, 128:160], Ms,
                                 start=True, stop=False, skip_group_check=True)
                nc.tensor.matmul(Gr2_ps[:, MT:GW], Ms[:, 128:160], _col(nqv_all, n),
                                 start=True, stop=True, skip_group_check=True)
                nc.vector.tensor_scalar_mul(t1(n), Gr1_ps, -1.0)
                nc.vector.tensor_scalar_mul(g2(n), Gr2_ps, -1.0)

                Htile = work.tile([128, 256], F32, tag="H")
                nc.scalar.activation(Htile[:, 0:128], H_ps[:, 0:128], AFT.Copy)
                nc.scalar.activation(Htile[0:32, 128:256], H_ps[0:32, 128:256],
                                     AFT.Copy)
                nc.sync.dma_start(out=hsp_d[n, 0:128, :], in_=Htile[:, 0:128])
                nc.sync.dma_start(out=hsp_d[n, 128:160, :], in_=Htile[0:32, 128:256])

                if taps and n == 0:
                    nc.sync.dma_start(out=dbg_d[0, :, 0:128], in_=negK)
                    nc.sync.dma_start(out=dbg_d[1, :, 0:128], in_=X)
                    nc.sync.dma_start(out=dbg_d[2, :, 0:MT], in_=Ms)
                    nc.sync.dma_start(out=dbg_d[3, :, 0:GW], in_=t1(0))
                    nc.sync.dma_start(out=dbg_d[4, 0:32, 0:GW], in_=g2(0))

            # S_all copy (phase-A s columns done)
            nc.vector.tensor_copy(S_all, Sbank)

            # ---------------- s1 init + C' prepass ----------------
            # top block: psum accumulates d_i - u_i (s1), then + g0_i (C')
            S1T = pspool.tile([128, n_el], F32, tag="ps")
            nc.tensor.matmul(S1T, negI, u_i, start=True, stop=False,
                             skip_group_check=True)
            nc.tensor.matmul(S1T, negI, _strided_cols(T1_all, MT, GW, n_el),
                             start=False, stop=False, skip_group_check=True)
            nc.vector.tensor_copy(s_i[0], S1T)
            # gather packed d_e columns (-d_e) into contiguous [32, n_el] layout
            dme_t = gpool.tile([32, n_el], F32)
            for a in range(4):
                cnt = (n_el - a + 3) // 4
                if cnt == 0:
                    continue
                nc.sync.dma_start(
                    out=_strided_cols(dme_t, a, 4, cnt),
                    in_=_strided_cols(G2_all, MT, GW, cnt, part=(32 * a, 32 * a + 32)))
            S1E = pspool.tile([32, n_el], F32, tag="ps")
            nc.tensor.matmul(S1E, negI[0:32, 0:32], u_e2, start=True, stop=False,
                             skip_group_check=True)
            nc.tensor.matmul(S1E, negI[0:32, 0:32], dme_t, start=False, stop=False,
                             skip_group_check=True)
            nc.vector.tensor_copy(s_e[0][:, 0:n_el], S1E)
            nc.vector.tensor_scalar(out=s_e[0][:, n_el:2 * n_el], in0=S1E,
                                    scalar1=-1.0, scalar2=-EPS_,
                                    op0=ALU.mult, op1=ALU.add)

            # continue accumulating g0 terms into the same psums -> C'
            for n in range(n_el):
                nc.tensor.matmul(_col(S1T, n), t1(n)[:, 0:128], _col(nruC_top, n),
                                 start=False, stop=False, skip_group_check=True)
                a = n % 4
                nc.tensor.matmul(_col(S1T, n), g2(n)[:, 0:128],
                                 nruC_bot[32 * a:32 * a + 32, n:n + 1],
                                 start=False, stop=(n == n_el - 1),
                                 skip_group_check=True, tile_position=(32 * a, 0))
                nc.tensor.matmul(_col(S1E, n), t1(n)[:, 128:160], _col(nruC_top, n),
                                 start=False, stop=False, skip_group_check=True)
                nc.tensor.matmul(_col(S1E, n), g2(n)[:, 128:160],
                                 nruC_bot[32 * a:32 * a + 32, n:n + 1],
                                 start=False, stop=(n == n_el - 1),
                                 skip_group_check=True, tile_position=(32 * a, 0))
            nc.vector.tensor_copy(Cp_i, S1T)
            nc.vector.tensor_copy(Cp_e[:, 0:n_el], S1E)
            nc.vector.tensor_scalar(out=Cp_e[:, n_el:2 * n_el], in0=S1E,
                                    scalar1=-1.0, scalar2=-EPS_,
                                    op0=ALU.mult, op1=ALU.add)
            if taps:
                nc.sync.dma_start(out=dbg_d[5, :, 0:n_el], in_=Cp_i)
                nc.sync.dma_start(out=dbg_d[6, :, 0:n_el], in_=s_i[0])

            # ---------------- phase B: ADMM loop ----------------
            def half_iter(src, dst):
                """One ADMM update: state src -> dst (indices into ping-pong)."""
                nc.scalar.activation(B_i[src], s_i[src], AFT.Abs, scale=RHO)
                nc.scalar.activation(B_e[src], s_e[src], AFT.Abs, scale=RHO)
                nc.vector.tensor_tensor(pbot[src][0:32, :], B_e[src][:, 0:n_el],
                                        B_e[src][:, n_el:2 * n_el], ALU.subtract)
                nc.vector.tensor_copy(pbot[src][32:64, :], pbot[src][0:32, :])
                nc.vector.tensor_copy(pbot[src][64:128, :], pbot[src][0:64, :])

                bankT = pspool.tile([128, n_el], F32, tag="ps")
                bankE = pspool.tile([32, 3 * n_el], F32, tag="ps")
                nc.tensor.matmul(bankT, ident, Cp_i, start=True, stop=False,
                                 skip_group_check=True)
                nc.tensor.matmul(bankT, hbrI, B_i[src], start=False, stop=False,
                                 skip_group_check=True)
                nc.tensor.matmul(bankT, halfI, s_i[src], start=False, stop=False,
                                 skip_group_check=True)
                nc.tensor.matmul(bankE[:, n_el:3 * n_el], ident[0:32, 0:32], Cp_e,
                                 start=True, stop=False, skip_group_check=True)
                nc.tensor.matmul(bankE[:, n_el:3 * n_el], hbrI[0:32, 0:32], B_e[src],
                                 start=False, stop=False, skip_group_check=True)
                nc.tensor.matmul(bankE[:, n_el:3 * n_el], halfI[0:32, 0:32], s_e[src],
                                 start=False, stop=False, skip_group_check=True)
                for n in range(n_el):
                    a = n % 4
                    last = n == n_el - 1
                    nc.tensor.matmul(_col(bankT, n), t1(n)[:, 0:128],
                                     _col(B_i[src], n), start=False, stop=False,
                                     skip_group_check=True)
                    nc.tensor.matmul(_col(bankT, n), g2(n)[:, 0:128],
                                     pbot[src][32 * a:32 * a + 32, n:n + 1],
                                     start=False, stop=last,
                                     skip_group_check=True, tile_position=(32 * a, 0))
                    nc.tensor.matmul(_col(bankE, n), t1(n)[:, 128:160],
                                     _col(B_i[src], n), start=True, stop=False,
                                     skip_group_check=True)
                    nc.tensor.matmul(_col(bankE, n), g2(n)[:, 128:160],
                                     pbot[src][32 * a:32 * a + 32, n:n + 1],
                                     start=False, stop=last,
                                     skip_group_check=True, tile_position=(32 * a, 0))
                nc.vector.tensor_copy(s_i[dst], bankT)
                nc.scalar.activation(he_sb[src], bankE[:, 0:n_el], AFT.Copy)
                nc.vector.tensor_tensor(s_e[dst][:, 0:n_el],
                                        bankE[:, n_el:2 * n_el],
                                        he_sb[src], ALU.add)
                nc.vector.tensor_tensor(s_e[dst][:, n_el:2 * n_el],
                                        bankE[:, 2 * n_el:3 * n_el],
                                        he_sb[src], ALU.subtract)

            if n_body > 0:
                with tc.For_i(0, n_body, 1,
                              hint_engines=(mybir.EngineType.PE,)):
                    half_iter(0, 1)
                    half_iter(1, 0)

            # ---------------- final: x = M (rho uC - p~_99) + s_vec -------------
            nc.scalar.activation(B_i[0], s_i[0], AFT.Abs, scale=RHO)
            nc.scalar.activation(B_e[0], s_e[0], AFT.Abs, scale=RHO)
            nc.vector.tensor_tensor(pbot[0][0:32, :], B_e[0][:, 0:n_el],
                                    B_e[0][:, n_el:2 * n_el], ALU.subtract)
            nc.vector.tensor_tensor(f_top, ruC_top, B_i[0], ALU.subtract)
            nc.vector.tensor_tensor(f_bot, ruC_bot, pbot[0][0:32, :], ALU.subtract)

            xP = pspool.tile([128, n_el], F32, tag="ps")
            nc.tensor.matmul(xP, ident, S_all, start=True, stop=False,
                             skip_group_check=True)
            for n in range(n_el):
                Ht = hre.tile([128, 128], F32, tag="hret")
                nc.sync.dma_start(out=Ht, in_=hsp_d[n, 0:128, :])
                Hb = hre.tile([32, 128], F32, tag="hreb")
                nc.sync.dma_start(out=Hb, in_=hsp_d[n, 128:160, :])
                nc.tensor.matmul(_col(xP, n), Ht, _col(f_top, n),
                                 start=False, stop=False, skip_group_check=True)
                nc.tensor.matmul(_col(xP, n), Hb, _col(f_bot, n),
                                 start=False, stop=(n == n_el - 1),
                                 skip_group_check=True)
            nc.vector.tensor_copy(xo, xP)
            if taps:
                nc.sync.dma_start(out=dbg_d[7, :, 0:n_el], in_=s_i[0])
            xT = pspool.tile([n_el, 128], F32, tag="ps")
            nc.tensor.transpose(xT, xo, ident)
            nc.vector.tensor_copy(xout, xT)
            nc.sync.dma_start(out=out_d[0:n_el, :, 0], in_=xout)

    nc.compile()
    return nc


_NC_CACHE = {}


def _get_nc(taps=False):
    key = taps
    if key not in _NC_CACHE:
        _NC_CACHE[key] = build(taps=taps)
    return _NC_CACHE[key]


def run(inputs, taps=False, trace=False):
    nc = _get_nc(taps=taps)
    in_maps = []
    for c in range(NCORES):
        sl = slice(c * P, (c + 1) * P)
        in_maps.append({k: np.ascontiguousarray(np.asarray(v)[sl], dtype=np.float32)
                        for k, v in inputs.items()})
    res = run_bass_kernel_spmd(nc, in_maps, core_ids=list(range(NCORES)),
                               trace=trace)
    out = np.concatenate([res.results[c]["out"] for c in range(NCORES)], axis=0)
    return out, res


def kernel(**inputs):
    out, _ = run(inputs)
    return out


# revision 15
# speedup vs baseline: 2.1680x; 2.1680x over previous
"""Trainium2 Bass kernel for nn_BackwardStep_38749194944853.

Batched ADMM QP solve (OSQP-style), N=1024 independent QPs of dim nx=128 with
mi=128 inequality + me=32 doubled equality constraints, 100 fixed iterations.

Strategy (pure data-parallel over 8 cores, 128 QPs per core):
  Phase A (per element, TensorE-heavy):
    K = Q + (1+sigma) I + rho (Ai'Ai + 2 Ae'Ae)
    Kinv via Newton-Schulz (scalar init c*I; K >= 1.1 I by construction)
    M = Kinv At'  (At = [Ai; Ae], 160x128);  G = At M (160x160, symmetric)
    d = At (-Kinv qv)  -> persistent fp32 PSUM bank;  s_vec = -Kinv qv
    Stationary tiles stored bf16: T1 = -G[0:128, 0:160], T2 = -G[128:160, 0:160]
    H = At Kinv (=M^T) spilled to DRAM (fp32) for the final matvec.
  Phase B (98 iterations), state s_t = a_t - u in blocks [i(128); e2(32); e3(32)],
  laid out [m-partitions, element-columns]:
    B   = |rho s|  (fp32 for the exact relu path; bf16 copy feeds the matvec)
    p~  = [B_i ; B_e2-B_e3]
    s' = C' + (0.5/rho) B + 0.5 s - G p~   (+G p~ for the e3 block)
    PSUM accumulates batched identity-stationary matmuls (affine terms, fp32)
    + per-element 4 bf16 matmuls with the stored -G tiles.
  Final: x = M (rho uC - p~_99) + s_vec via the spilled fp32 H as stationary.

Numerics validated vs fp64 replica of the reference: rel err ~2.8e-3 (bf16 G).
"""
import os
import numpy as np

import concourse.bass as bass
import concourse.bacc as bacc
import concourse.mybir as mybir
from concourse.tile import TileContext
from concourse.masks import make_identity
from concourse.bass_utils import run_bass_kernel_spmd

F32 = mybir.dt.float32
BF16 = mybir.dt.bfloat16
ALU = mybir.AluOpType
AFT = mybir.ActivationFunctionType

NCORES = 8
P = 128            # elements per core
NX = 128           # QP dimension
MI = 128           # inequality rows
ME = 32            # equality rows
MT = MI + ME       # 160 collapsed constraint dim

RHO = 0.1
EPS_ = 1e-4
ACOEF = 1.0 + 1e-6          # alpha + sigma added to Q's diagonal
C0 = float(2.0 / (1.1 + 7.5))  # Newton-Schulz scalar init
NS_LOOP = 7                  # NS iterations after the fused first one (8 total)
N_ITER = 100                 # reference ADMM iterations
N_AUPD = N_ITER - 2          # 98 a-state updates (a_1 given, w from a_99)
N_BODY = N_AUPD // 2         # 49 For_i bodies x 2 updates


def _col(t, n):
    return t[:, n:n + 1]


def build(n_el=P, n_body=N_BODY, ns_loop=NS_LOOP, taps=False):
    nc = bacc.Bacc()

    x_d = nc.dram_tensor("x", [P, NX, 1], F32, kind="ExternalInput")
    Q_d = nc.dram_tensor("Q", [P, NX, NX], F32, kind="ExternalInput")
    q_d = nc.dram_tensor("q", [P, NX, 1], F32, kind="ExternalInput")
    Ai_d = nc.dram_tensor("A_ineq", [P, MI, NX], F32, kind="ExternalInput")
    bi_d = nc.dram_tensor("b_ineq", [P, MI, 1], F32, kind="ExternalInput")
    Ae_d = nc.dram_tensor("A_eq", [P, ME, NX], F32, kind="ExternalInput")
    be_d = nc.dram_tensor("b_eq", [P, ME, 1], F32, kind="ExternalInput")
    out_d = nc.dram_tensor("out", [P, NX, 1], F32, kind="ExternalOutput")
    hsp_d = nc.dram_tensor("hspill", [P, MT, NX], F32)  # internal DRAM
    if taps:
        dbg_d = nc.dram_tensor("dbg", [8, 128, 256], F32, kind="ExternalOutput")

    with TileContext(nc) as tc:
        with (
            tc.tile_pool(name="consts", bufs=1) as consts,
            tc.tile_pool(name="gpool", bufs=1) as gpool,
            tc.tile_pool(name="work", bufs=3) as work,
            tc.tile_pool(name="wks", bufs=2) as wks,
            tc.tile_pool(name="hre", bufs=8) as hre,
            tc.tile_pool(name="pspool", bufs=6, space="PSUM") as pspool,
            tc.tile_pool(name="sbpool", bufs=1, space="PSUM") as sbpool,
        ):
            # ---------------- constants ----------------
            ident = consts.tile([128, 128], F32)
            make_identity(nc, ident)
            negI = consts.tile([128, 128], F32)
            nc.vector.tensor_scalar_mul(negI, ident, -1.0)
            halfI = consts.tile([128, 128], F32)
            nc.vector.tensor_scalar_mul(halfI, ident, 0.5)
            hbrI = consts.tile([128, 128], F32)
            nc.vector.tensor_scalar_mul(hbrI, ident, 0.5 / RHO)
            twoI = consts.tile([128, 128], F32)
            nc.vector.tensor_scalar_mul(twoI, ident, 2.0)
            twoCid = consts.tile([128, 128], F32)
            nc.vector.tensor_scalar_mul(twoCid, ident, 2.0 * C0)
            cIdent = consts.tile([128, 128], F32)
            nc.vector.tensor_scalar_mul(cIdent, ident, ACOEF)

            # ---------------- persistent big tiles ----------------
            # T1_all: per element -G[0:128, 0:160] bf16, [128, n_el*160]
            T1_all = gpool.tile([128, n_el * MT], BF16)
            # G2_all: per element -G[128:160, 0:160] bf16, packed 4 elements per
            # partition group: element n at partitions 32*(n%4), slice n//4.
            G2_all = gpool.tile([128, (n_el // 4) * MT], BF16)

            def t1(n):
                return T1_all[:, n * MT:(n + 1) * MT]

            def g2(n):
                a, g = n % 4, n // 4
                return G2_all[32 * a:32 * a + 32, g * MT:(g + 1) * MT]

            # batched constants (m-layout: [m-part, element-cols])
            u_i = gpool.tile([128, n_el], F32)
            be_t = gpool.tile([32, n_el], F32)
            u_e2 = gpool.tile([32, n_el], F32)
            ruC_top = gpool.tile([128, n_el], F32)
            ruC_bot = gpool.tile([32, n_el], F32)
            nruC_top = gpool.tile([128, n_el], BF16)
            nruC_bot = gpool.tile([128, n_el], BF16)  # replicated x4
            nqv_all = gpool.tile([128, n_el], F32)
            Cp_i = gpool.tile([128, n_el], F32)
            Cp_e = gpool.tile([32, 2 * n_el], F32)    # [Cp_e2 | Cp_e3]
            S_all = gpool.tile([128, n_el], F32)
            D_all = gpool.tile([128, 2 * n_el], F32)  # [d_top | d_bot(32p)]
            # ADMM state (ping-pong a/b)
            s_i = [gpool.tile([128, n_el], F32, name=f"s_i{j}") for j in range(2)]
            s_e = [gpool.tile([32, 2 * n_el], F32, name=f"s_e{j}") for j in range(2)]
            B_i = [gpool.tile([128, n_el], F32, name=f"B_i{j}") for j in range(2)]
            B_e = [gpool.tile([32, 2 * n_el], F32, name=f"B_e{j}") for j in range(2)]
            Bib = [gpool.tile([128, n_el], BF16, name=f"Bib{j}") for j in range(2)]
            pbot = [gpool.tile([128, n_el], BF16, name=f"pbot{j}") for j in range(2)]
            he_sb = [gpool.tile([32, n_el], F32, name=f"he_sb{j}") for j in range(2)]
            f_top = gpool.tile([128, n_el], F32)
            f_bot = gpool.tile([32, n_el], F32)
            xo = gpool.tile([128, n_el], F32)
            xout = gpool.tile([n_el, 128], F32)

            Sbank = sbpool.tile([128, n_el], F32, tag="sbank")
            Dbank = sbpool.tile([128, 2 * n_el], F32, tag="dbank")

            # ---------------- batched input prep ----------------
            x_el = wks.tile([P, NX], F32, tag="xel")
            q_el = wks.tile([P, NX], F32, tag="qel")
            nc.sync.dma_start(out=x_el, in_=x_d[:, :, 0])
            nc.sync.dma_start(out=q_el, in_=q_d[:, :, 0])
            nq_el = wks.tile([P, NX], F32, tag="nqel")
            nc.vector.tensor_tensor(nq_el, x_el, q_el, ALU.subtract)  # -(q - x)
            nqps = pspool.tile([128, P], F32, tag="ps")
            nc.tensor.transpose(nqps, nq_el, ident)
            nc.vector.tensor_copy(nqv_all, nqps[:, 0:n_el])

            bi_el = wks.tile([P, MI], F32, tag="biel")
            nc.sync.dma_start(out=bi_el, in_=bi_d[:, :, 0])
            bips = pspool.tile([128, P], F32, tag="ps")
            nc.tensor.transpose(bips, bi_el, ident)
            nc.vector.tensor_copy(u_i, bips[:, 0:n_el])

            be_el = wks.tile([P, ME], F32, tag="beel")
            nc.sync.dma_start(out=be_el, in_=be_d[:, :, 0])
            beps = pspool.tile([32, P], F32, tag="ps")
            nc.tensor.transpose(beps, be_el, ident)
            nc.vector.tensor_copy(be_t, beps[:, 0:n_el])

            nc.vector.tensor_scalar_add(u_e2, be_t, EPS_)
            nc.vector.tensor_scalar_mul(ruC_top, u_i, RHO)
            nc.vector.tensor_scalar(out=ruC_bot, in0=be_t, scalar1=2.0 * RHO,
                                    scalar2=RHO * EPS_, op0=ALU.mult, op1=ALU.add)
            nc.vector.tensor_scalar_mul(nruC_top, u_i, -RHO)
            nc.vector.tensor_scalar(out=nruC_bot[0:32, :], in0=be_t,
                                    scalar1=-2.0 * RHO, scalar2=-RHO * EPS_,
                                    op0=ALU.mult, op1=ALU.add)
            nc.vector.tensor_copy(nruC_bot[32:64, :], nruC_bot[0:32, :])
            nc.vector.tensor_copy(nruC_bot[64:128, :], nruC_bot[0:64, :])

            # ---------------- phase A: per-element factorization ----------------
            for n in range(n_el):
                Qt = work.tile([128, 128], F32, tag="Q")
                nc.sync.dma_start(out=Qt, in_=Q_d[n])
                Ait = work.tile([128, 128], F32, tag="Ai")
                nc.sync.dma_start(out=Ait, in_=Ai_d[n])
                Aet = work.tile([32, 128], F32, tag="Ae")
                nc.sync.dma_start(out=Aet, in_=Ae_d[n])

                at_ps = pspool.tile([128, MT], F32, tag="ps")
                nc.tensor.transpose(at_ps[:, 0:128], Ait, ident)
                nc.tensor.transpose(at_ps[:, 128:160], Aet, ident[0:32, 0:32])
                AT = work.tile([128, MT], F32, tag="AT")
                nc.vector.tensor_copy(AT, at_ps)

                AiS = work.tile([128, 128], F32, tag="AiS")
                nc.vector.tensor_scalar_mul(AiS, Ait, RHO)
                AeS = work.tile([32, 128], F32, tag="AeS")
                nc.scalar.activation(AeS, Aet, AFT.Copy, scale=2.0 * RHO)

                K_ps = pspool.tile([128, 128], F32, tag="ps")
                nc.tensor.matmul(K_ps, Ait, AiS, start=True, stop=False)
                nc.tensor.matmul(K_ps, Aet, AeS, start=False, stop=False)
                nc.tensor.matmul(K_ps, ident, Qt, start=False, stop=True)
                negK = work.tile([128, 128], F32, tag="negK")
                nc.vector.scalar_tensor_tensor(out=negK, in0=K_ps, scalar=-1.0,
                                               in1=cIdent, op0=ALU.mult,
                                               op1=ALU.subtract)
                X = work.tile([128, 128], F32, tag="X")
                nc.vector.scalar_tensor_tensor(out=X, in0=negK, scalar=C0 * C0,
                                               in1=twoCid, op0=ALU.mult,
                                               op1=ALU.add)
                for _ in range(ns_loop):
                    G1_ps = pspool.tile([128, 128], F32, tag="ps")
                    nc.tensor.matmul(G1_ps, negK, X, start=True, stop=True)
                    g1 = work.tile([128, 128], F32, tag="g1")
                    nc.scalar.activation(g1, G1_ps, AFT.Copy)
                    X2_ps = pspool.tile([128, 128], F32, tag="ps")
                    nc.tensor.matmul(X2_ps, X, g1, start=True, stop=False)
                    nc.tensor.matmul(X2_ps, twoI, X, start=False, stop=True)
                    Xn = work.tile([128, 128], F32, tag="X")
                    nc.vector.tensor_copy(Xn, X2_ps)
                    X = Xn

                # M = Kinv At' ; s_vec column into persistent Sbank
                Ms_ps = pspool.tile([128, MT], F32, tag="ps")
                nc.tensor.matmul(Ms_ps, X, AT, start=True, stop=True)
                nc.tensor.matmul(_col(Sbank, n), X, _col(nqv_all, n),
                                 start=True, stop=True, skip_group_check=True)
                Ms = work.tile([128, MT], F32, tag="Ms")
                nc.vector.tensor_copy(Ms, Ms_ps)

                # d columns (fp32) into persistent Dbank: d = M^T nqv = H nqv
                nc.tensor.matmul(_col(Dbank, n), Ms[:, 0:128], _col(nqv_all, n),
                                 start=True, stop=True, skip_group_check=True)
                nc.tensor.matmul(Dbank[0:32, n_el + n:n_el + n + 1],
                                 Ms[:, 128:160], _col(nqv_all, n),
                                 start=True, stop=True, skip_group_check=True)

                # H = At Kinv  (two chunks into one psum bank)
                H_ps = pspool.tile([128, 256], F32, tag="ps")
                nc.tensor.matmul(H_ps[:, 0:128], AT[:, 0:128], X,
                                 start=True, stop=False, skip_group_check=True)
                nc.tensor.matmul(H_ps[0:32, 128:256], AT[:, 128:160], X,
                                 start=False, stop=True, skip_group_check=True)

                # G rows -> bf16 tiles (negated)
                Gr1_ps = pspool.tile([128, MT], F32, tag="ps")
                nc.tensor.matmul(Gr1_ps, AT[:, 0:128], Ms, start=True, stop=True)
                Gr2_ps = pspool.tile([32, MT], F32, tag="ps")
                nc.tensor.matmul(Gr2_ps, AT[:, 128:160], Ms, start=True, stop=True)
                nc.vector.tensor_scalar_mul(t1(n), Gr1_ps, -1.0)
                nc.vector.tensor_scalar_mul(g2(n), Gr2_ps, -1.0)

                Htile = work.tile([128, 256], F32, tag="H")
                nc.scalar.activation(Htile[:, 0:128], H_ps[:, 0:128], AFT.Copy)
                nc.scalar.activation(Htile[0:32, 128:256], H_ps[0:32, 128:256],
                                     AFT.Copy)
                nc.sync.dma_start(out=hsp_d[n, 0:128, :], in_=Htile[:, 0:128])
                nc.sync.dma_start(out=hsp_d[n, 128:160, :], in_=Htile[0:32, 128:256])

                if taps and n == 0:
                    nc.sync.dma_start(out=dbg_d[0, :, 0:128], in_=negK)
                    nc.sync.dma_start(out=dbg_d[1, :, 0:128], in_=X)
                    nc.sync.dma_start(out=dbg_d[2, :, 0:MT], in_=Ms)

            nc.vector.tensor_copy(S_all, Sbank)
            nc.vector.tensor_copy(D_all[:, 0:n_el], Dbank[:, 0:n_el])
            nc.vector.tensor_copy(D_all[0:32, n_el:2 * n_el],
                                  Dbank[0:32, n_el:2 * n_el])

            # ---------------- s1 init + C' prepass ----------------
            # top: psum = d_i - u_i (s1), then + g0_i (C')
            S1T = pspool.tile([128, n_el], F32, tag="ps")
            nc.tensor.matmul(S1T, negI, u_i, start=True, stop=False,
                             skip_group_check=True)
            nc.tensor.matmul(S1T, ident, D_all[:, 0:n_el], start=False, stop=False,
                             skip_group_check=True)
            nc.vector.tensor_copy(s_i[0], S1T)
            S1E = pspool.tile([32, n_el], F32, tag="ps")
            nc.tensor.matmul(S1E, negI[0:32, 0:32], u_e2, start=True, stop=False,
                             skip_group_check=True)
            nc.tensor.matmul(S1E, ident[0:32, 0:32],
                             D_all[0:32, n_el:2 * n_el], start=False, stop=False,
                             skip_group_check=True)
            nc.vector.tensor_copy(s_e[0][:, 0:n_el], S1E)
            nc.vector.tensor_scalar(out=s_e[0][:, n_el:2 * n_el], in0=S1E,
                                    scalar1=-1.0, scalar2=-EPS_,
                                    op0=ALU.mult, op1=ALU.add)

            # accumulate g0 terms (bf16 G x bf16 -rho*uC) into the same psums
            for n in range(n_el):
                a = n % 4
                last = n == n_el - 1
                nc.tensor.matmul(_col(S1T, n), t1(n)[:, 0:128], _col(nruC_top, n),
                                 start=False, stop=False, skip_group_check=True)
                nc.tensor.matmul(_col(S1T, n), g2(n)[:, 0:128],
                                 nruC_bot[32 * a:32 * a + 32, n:n + 1],
                                 start=False, stop=last,
                                 skip_group_check=True, tile_position=(32 * a, 0))
                nc.tensor.matmul(_col(S1E, n), t1(n)[:, 128:160], _col(nruC_top, n),
                                 start=False, stop=False, skip_group_check=True)
                nc.tensor.matmul(_col(S1E, n), g2(n)[:, 128:160],
                                 nruC_bot[32 * a:32 * a + 32, n:n + 1],
                                 start=False, stop=last,
                                 skip_group_check=True, tile_position=(32 * a, 0))
            nc.vector.tensor_copy(Cp_i, S1T)
            nc.vector.tensor_copy(Cp_e[:, 0:n_el], S1E)
            nc.vector.tensor_scalar(out=Cp_e[:, n_el:2 * n_el], in0=S1E,
                                    scalar1=-1.0, scalar2=-EPS_,
                                    op0=ALU.mult, op1=ALU.add)
            if taps:
                nc.sync.dma_start(out=dbg_d[5, :, 0:n_el], in_=Cp_i)
                nc.sync.dma_start(out=dbg_d[6, :, 0:n_el], in_=s_i[0])

            # ---------------- phase B: ADMM loop ----------------
            def half_iter(src, dst):
                nc.scalar.activation(B_i[src], s_i[src], AFT.Abs, scale=RHO)
                nc.scalar.activation(B_e[src], s_e[src], AFT.Abs, scale=RHO)
                nc.scalar.activation(Bib[src], B_i[src], AFT.Copy)
                nc.vector.tensor_tensor(pbot[src][0:32, :], B_e[src][:, 0:n_el],
                                        B_e[src][:, n_el:2 * n_el], ALU.subtract)
                nc.vector.tensor_copy(pbot[src][32:64, :], pbot[src][0:32, :])
                nc.vector.tensor_copy(pbot[src][64:128, :], pbot[src][0:64, :])

                bankT = pspool.tile([128, n_el], F32, tag="ps")
                bankE = pspool.tile([32, 3 * n_el], F32, tag="ps")
                nc.tensor.matmul(bankT, ident, Cp_i, start=True, stop=False,
                                 skip_group_check=True)
                nc.tensor.matmul(bankT, hbrI, B_i[src], start=False, stop=False,
                                 skip_group_check=True)
                nc.tensor.matmul(bankT, halfI, s_i[src], start=False, stop=False,
                                 skip_group_check=True)
                nc.tensor.matmul(bankE[:, n_el:3 * n_el], ident[0:32, 0:32], Cp_e,
                                 start=True, stop=False, skip_group_check=True)
                nc.tensor.matmul(bankE[:, n_el:3 * n_el], hbrI[0:32, 0:32], B_e[src],
                                 start=False, stop=False, skip_group_check=True)
                nc.tensor.matmul(bankE[:, n_el:3 * n_el], halfI[0:32, 0:32], s_e[src],
                                 start=False, stop=False, skip_group_check=True)
                for n in range(n_el):
                    a = n % 4
                    last = n == n_el - 1
                    nc.tensor.matmul(_col(bankT, n), t1(n)[:, 0:128],
                                     _col(Bib[src], n), start=False, stop=False,
                                     skip_group_check=True)
                    nc.tensor.matmul(_col(bankT, n), g2(n)[:, 0:128],
                                     pbot[src][32 * a:32 * a + 32, n:n + 1],
                                     start=False, stop=last,
                                     skip_group_check=True, tile_position=(32 * a, 0))
                    nc.tensor.matmul(_col(bankE, n), t1(n)[:, 128:160],
                                     _col(Bib[src], n), start=True, stop=False,
                                     skip_group_check=True)
                    nc.tensor.matmul(_col(bankE, n), g2(n)[:, 128:160],
                                     pbot[src][32 * a:32 * a + 32, n:n + 1],
                                     start=False, stop=last,
                                     skip_group_check=True, tile_position=(32 * a, 0))
                nc.vector.tensor_copy(s_i[dst], bankT)
                nc.scalar.activation(he_sb[src], bankE[:, 0:n_el], AFT.Copy)
                nc.vector.tensor_tensor(s_e[dst][:, 0:n_el],
                                        bankE[:, n_el:2 * n_el],
                                        he_sb[src], ALU.add)
                nc.vector.tensor_tensor(s_e[dst][:, n_el:2 * n_el],
                                        bankE[:, 2 * n_el:3 * n_el],
                                        he_sb[src], ALU.subtract)

            if n_body > 0:
                with tc.For_i(0, n_body, 1,
                              hint_engines=(mybir.EngineType.PE,)):
                    half_iter(0, 1)
                    half_iter(1, 0)

            # ---------------- final: x = M (rho uC - p~_99) + s_vec -------------
            nc.scalar.activation(B_i[0], s_i[0], AFT.Abs, scale=RHO)
            nc.scalar.activation(B_e[0], s_e[0], AFT.Abs, scale=RHO)
            nc.vector.tensor_tensor(f_bot, B_e[0][:, 0:n_el],
                                    B_e[0][:, n_el:2 * n_el], ALU.subtract)
            nc.vector.tensor_tensor(f_bot, ruC_bot, f_bot, ALU.subtract)
            nc.vector.tensor_tensor(f_top, ruC_top, B_i[0], ALU.subtract)

            xP = pspool.tile([128, n_el], F32, tag="ps")
            nc.tensor.matmul(xP, ident, S_all, start=True, stop=False,
                             skip_group_check=True)
            for n in range(n_el):
                Ht = hre.tile([128, 128], F32, tag="hret")
                nc.sync.dma_start(out=Ht, in_=hsp_d[n, 0:128, :])
                Hb = hre.tile([32, 128], F32, tag="hreb")
                nc.sync.dma_start(out=Hb, in_=hsp_d[n, 128:160, :])
                nc.tensor.matmul(_col(xP, n), Ht, _col(f_top, n),
                                 start=False, stop=False, skip_group_check=True)
                nc.tensor.matmul(_col(xP, n), Hb, _col(f_bot, n),
                                 start=False, stop=(n == n_el - 1),
                                 skip_group_check=True)
            nc.vector.tensor_copy(xo, xP)
            if taps:
                nc.sync.dma_start(out=dbg_d[7, :, 0:n_el], in_=s_i[0])
            xT = pspool.tile([n_el, 128], F32, tag="ps")
            nc.tensor.transpose(xT, xo, ident)
            nc.vector.tensor_copy(xout, xT)
            nc.sync.dma_start(out=out_d[0:n_el, :, 0], in_=xout)

    nc.compile()
    return nc


_NC_CACHE = {}


def _get_nc(taps=False):
    key = taps
    if key not in _NC_CACHE:
        _NC_CACHE[key] = build(taps=taps)
    return _NC_CACHE[key]


def run(inputs, taps=False, trace=False):
    nc = _get_nc(taps=taps)
    in_maps = []
    for c in range(NCORES):
        sl = slice(c * P, (c + 1) * P)
        in_maps.append({k: np.ascontiguousarray(np.asarray(v)[sl], dtype=np.float32)
                        for k, v in inputs.items()})
    res = run_bass_kernel_spmd(nc, in_maps, core_ids=list(range(NCORES)),
                               trace=trace)
    out = np.concatenate([res.results[c]["out"] for c in range(NCORES)], axis=0)
    return out, res


def kernel(**inputs):
    out, _ = run(inputs)
    return out


# revision 29
# speedup vs baseline: 3.0646x; 1.4136x over previous
"""Trainium2 Bass kernel for nn_BackwardStep_38749194944853.

Batched ADMM QP solve (OSQP-style), N=1024 independent QPs of dim nx=128 with
mi=128 inequality + me=32 doubled equality constraints, 100 fixed iterations.

Strategy (pure data-parallel over 8 cores, 128 QPs per core):
  Phase A (per element, TensorE-heavy):
    K = Q + (1+sigma) I + rho (Ai'Ai + 2 Ae'Ae)
    Kinv via Newton-Schulz (scalar init c*I; K >= 1.1 I by construction)
    M = Kinv At'  (At = [Ai; Ae], 160x128);  G = At M (160x160, symmetric)
    d = At (-Kinv qv)  -> persistent fp32 PSUM bank;  s_vec = -Kinv qv
    Stationary tiles stored bf16: T1 = -G[0:128, 0:160], T2 = -G[128:160, 0:160]
    H = At Kinv (=M^T) spilled to DRAM (fp32) for the final matvec.
  Phase B (98 iterations), state s_t = a_t - u in blocks [i(128); e2(32); e3(32)],
  laid out [m-partitions, element-columns]:
    B   = |rho s|  (fp32 for the exact relu path; bf16 copy feeds the matvec)
    p~  = [B_i ; B_e2-B_e3]
    s' = C' + (0.5/rho) B + 0.5 s - G p~   (+G p~ for the e3 block)
    PSUM accumulates batched identity-stationary matmuls (affine terms, fp32)
    + per-element 4 bf16 matmuls with the stored -G tiles.
  Final: x = M (rho uC - p~_99) + s_vec via the spilled fp32 H as stationary.

Numerics validated vs fp64 replica of the reference: rel err ~2.8e-3 (bf16 G).
"""
import os
import numpy as np

import concourse.bass as bass
import concourse.bacc as bacc
import concourse.mybir as mybir
from concourse.tile import TileContext
from concourse.masks import make_identity
from concourse.bass_utils import run_bass_kernel_spmd

F32 = mybir.dt.float32
BF16 = mybir.dt.bfloat16
ALU = mybir.AluOpType
AFT = mybir.ActivationFunctionType

NCORES = 8
P = 128            # elements per core
NX = 128           # QP dimension
MI = 128           # inequality rows
ME = 32            # equality rows
MT = MI + ME       # 160 collapsed constraint dim

RHO = 0.1
EPS_ = 1e-4
ACOEF = 1.0 + 1e-6          # alpha + sigma added to Q's diagonal
C0 = float(2.0 / (1.1 + 7.5))  # Newton-Schulz scalar init
NS_LOOP = 7                  # NS iterations after the fused first one (8 total)
N_ITER = 100                 # reference ADMM iterations
N_AUPD = N_ITER - 2          # 98 a-state updates (a_1 given, w from a_99)
N_BODY = N_AUPD // 2         # 49 For_i bodies x 2 updates


def _col(t, n):
    return t[:, n:n + 1]


def _strided_cols(t, start, step, count, part=None):
    base = t[:, 0:1] if part is None else t[part[0]:part[1], 0:1]
    return bass.AP(tensor=base.tensor, offset=base.offset + start,
                   ap=[base.ap[0], [step, count]])


def build(n_el=P, n_body=N_BODY, ns_loop=NS_LOOP, taps=False):
    nc = bacc.Bacc()

    x_d = nc.dram_tensor("x", [P, NX, 1], F32, kind="ExternalInput")
    Q_d = nc.dram_tensor("Q", [P, NX, NX], F32, kind="ExternalInput")
    q_d = nc.dram_tensor("q", [P, NX, 1], F32, kind="ExternalInput")
    Ai_d = nc.dram_tensor("A_ineq", [P, MI, NX], F32, kind="ExternalInput")
    bi_d = nc.dram_tensor("b_ineq", [P, MI, 1], F32, kind="ExternalInput")
    Ae_d = nc.dram_tensor("A_eq", [P, ME, NX], F32, kind="ExternalInput")
    be_d = nc.dram_tensor("b_eq", [P, ME, 1], F32, kind="ExternalInput")
    out_d = nc.dram_tensor("out", [P, NX, 1], F32, kind="ExternalOutput")
    hsp_d = nc.dram_tensor("hspill", [P, MT, NX], F32)  # internal DRAM
    if taps:
        dbg_d = nc.dram_tensor("dbg", [8, 128, 256], F32, kind="ExternalOutput")

    with TileContext(nc) as tc:
        with (
            tc.tile_pool(name="consts", bufs=1) as consts,
            tc.tile_pool(name="gpool", bufs=1) as gpool,
            tc.tile_pool(name="work", bufs=3) as work,
            tc.tile_pool(name="wks", bufs=2) as wks,
            tc.tile_pool(name="hre", bufs=8) as hre,
            tc.tile_pool(name="pspool", bufs=6, space="PSUM") as pspool,
            tc.tile_pool(name="sbpool", bufs=1, space="PSUM") as sbpool,
        ):
            # ---------------- constants ----------------
            ident = consts.tile([128, 128], F32)
            make_identity(nc, ident)
            negI = consts.tile([128, 128], F32)
            nc.vector.tensor_scalar_mul(negI, ident, -1.0)
            halfI = consts.tile([128, 128], F32)
            nc.vector.tensor_scalar_mul(halfI, ident, 0.5)
            hbrI = consts.tile([128, 128], F32)
            nc.vector.tensor_scalar_mul(hbrI, ident, 0.5 / RHO)
            twoI = consts.tile([128, 128], F32)
            nc.vector.tensor_scalar_mul(twoI, ident, 2.0)
            twoCid = consts.tile([128, 128], F32)
            nc.vector.tensor_scalar_mul(twoCid, ident, 2.0 * C0)
            cIdent = consts.tile([128, 128], F32)
            nc.vector.tensor_scalar_mul(cIdent, ident, ACOEF)
            xinitI = consts.tile([128, 128], F32)
            nc.vector.tensor_scalar_mul(xinitI, ident, 2.0 * C0 - C0 * C0 * ACOEF)
            twoIb = consts.tile([128, 128], BF16)
            nc.vector.tensor_scalar_mul(twoIb, ident, 2.0)

            # ---------------- persistent big tiles ----------------
            Q = n_el // 4  # quads: element n = 4q+k at partition block 32k
            # T1_all: per element -G[0:128, 0:160] bf16, [128, n_el*160]
            T1_all = gpool.tile([128, n_el * MT], BF16)
            # G2A_all: quad-stacked -G[128:160, 0:128] chunks: element 4q+k at
            # partitions 32k, cols q*128..; used as one [128,128] lhsT per quad
            # with a block-sparse rhs.
            G2A_all = gpool.tile([128, Q * 128], BF16)
            # G2e_all: per element -G[128:160, 128:160] blocks, packed 4 per
            # partition group (element n at partitions 32*(n%4), col n//4*32).
            G2e_all = gpool.tile([128, Q * 32], BF16)

            def t1(n):
                return T1_all[:, n * MT:(n + 1) * MT]

            def g2e(n):
                a, g = n % 4, n // 4
                return G2e_all[32 * a:32 * a + 32, g * 32:(g + 1) * 32]

            # batched constants (m-layout: [m-part, element-cols])
            u_i = gpool.tile([128, n_el], F32)
            be_t = gpool.tile([32, n_el], F32)
            u_e2 = gpool.tile([32, n_el], F32)
            ruC_top = gpool.tile([128, n_el], F32)
            ruC_bot = gpool.tile([32, n_el], F32)
            nruC_top = gpool.tile([128, n_el], BF16)
            nruC_bot = gpool.tile([128, n_el], BF16)   # replicated x4
            nruC_botD = gpool.tile([128, n_el], BF16)  # block-sparse diag scatter
            nqv_all = gpool.tile([128, n_el], F32)
            Cp_i = gpool.tile([128, n_el], F32)
            Cp_e = gpool.tile([32, 2 * n_el], F32)     # [Cp_e2 | Cp_e3]
            S_all = gpool.tile([128, n_el], F32)
            D_all = gpool.tile([128, 2 * n_el], F32)   # [d_top | d_bot(32p)]
            # ADMM state (ping-pong a/b)
            s_i = [gpool.tile([128, n_el], F32, name=f"s_i{j}") for j in range(2)]
            s_e = [gpool.tile([32, 2 * n_el], F32, name=f"s_e{j}") for j in range(2)]
            B_i = [gpool.tile([128, n_el], F32, name=f"B_i{j}") for j in range(2)]
            B_e = [gpool.tile([32, 2 * n_el], F32, name=f"B_e{j}") for j in range(2)]
            Bib = [gpool.tile([128, n_el], BF16, name=f"Bib{j}") for j in range(2)]
            pbot = [gpool.tile([128, n_el], BF16, name=f"pbot{j}") for j in range(2)]
            pbotD = [gpool.tile([128, n_el], BF16, name=f"pbotD{j}") for j in range(2)]
            he_sb = [gpool.tile([32, n_el], F32, name=f"he_sb{j}") for j in range(2)]
            f_top = gpool.tile([128, n_el], F32)
            f_bot = gpool.tile([32, n_el], F32)
            xo = gpool.tile([128, n_el], F32)
            xout = gpool.tile([n_el, 128], F32)

            Sbank = sbpool.tile([128, n_el], F32, tag="sbank")
            Dbank = sbpool.tile([128, 2 * n_el], F32, tag="dbank")

            nc.vector.memset(pbotD[0], 0.0)
            nc.vector.memset(pbotD[1], 0.0)
            nc.vector.memset(nruC_botD, 0.0)

            # ---------------- batched input prep ----------------
            x_el = wks.tile([P, NX], F32, tag="xel")
            q_el = wks.tile([P, NX], F32, tag="qel")
            nc.sync.dma_start(out=x_el, in_=x_d[:, :, 0])
            nc.sync.dma_start(out=q_el, in_=q_d[:, :, 0])
            nq_el = wks.tile([P, NX], F32, tag="nqel")
            nc.vector.tensor_tensor(nq_el, x_el, q_el, ALU.subtract)  # -(q - x)
            nqps = pspool.tile([128, P], F32, tag="ps")
            nc.tensor.transpose(nqps, nq_el, ident)
            nc.vector.tensor_copy(nqv_all, nqps[:, 0:n_el])

            bi_el = wks.tile([P, MI], F32, tag="biel")
            nc.sync.dma_start(out=bi_el, in_=bi_d[:, :, 0])
            bips = pspool.tile([128, P], F32, tag="ps")
            nc.tensor.transpose(bips, bi_el, ident)
            nc.vector.tensor_copy(u_i, bips[:, 0:n_el])

            be_el = wks.tile([P, ME], F32, tag="beel")
            nc.sync.dma_start(out=be_el, in_=be_d[:, :, 0])
            beps = pspool.tile([32, P], F32, tag="ps")
            nc.tensor.transpose(beps, be_el, ident)
            nc.vector.tensor_copy(be_t, beps[:, 0:n_el])

            nc.vector.tensor_scalar_add(u_e2, be_t, EPS_)
            nc.vector.tensor_scalar_mul(ruC_top, u_i, RHO)
            nc.vector.tensor_scalar(out=ruC_bot, in0=be_t, scalar1=2.0 * RHO,
                                    scalar2=RHO * EPS_, op0=ALU.mult, op1=ALU.add)
            nc.vector.tensor_scalar_mul(nruC_top, u_i, -RHO)
            nc.vector.tensor_scalar(out=nruC_bot[0:32, :], in0=be_t,
                                    scalar1=-2.0 * RHO, scalar2=-RHO * EPS_,
                                    op0=ALU.mult, op1=ALU.add)
            nc.vector.tensor_copy(nruC_bot[32:64, :], nruC_bot[0:32, :])
            nc.vector.tensor_copy(nruC_bot[64:128, :], nruC_bot[0:64, :])
            for k in range(4):
                nc.vector.tensor_copy(
                    _strided_cols(nruC_botD, k, 4, Q, part=(32 * k, 32 * k + 32)),
                    _strided_cols(nruC_bot, k, 4, Q, part=(32 * k, 32 * k + 32)))

            # ---------------- phase A: per-element factorization ----------------
            for n in range(n_el):
                Qt = work.tile([128, 128], F32, tag="Q")
                nc.sync.dma_start(out=Qt, in_=Q_d[n])
                Ait = work.tile([128, 128], F32, tag="Ai")
                nc.sync.dma_start(out=Ait, in_=Ai_d[n])
                Aet = work.tile([32, 128], F32, tag="Ae")
                nc.sync.dma_start(out=Aet, in_=Ae_d[n])

                at_ps = pspool.tile([128, MT], F32, tag="ps")
                nc.tensor.transpose(at_ps[:, 0:128], Ait, ident)
                nc.tensor.transpose(at_ps[:, 128:160], Aet, ident[0:32, 0:32])
                AT = work.tile([128, MT], F32, tag="AT")
                nc.vector.tensor_copy(AT, at_ps)

                Aib = work.tile([128, 128], BF16, tag="Aib")
                nc.scalar.activation(Aib, Ait, AFT.Copy)
                AiS = work.tile([128, 128], BF16, tag="AiS")
                nc.vector.tensor_scalar_mul(AiS, Ait, RHO)
                Aeb = work.tile([32, 128], BF16, tag="Aeb")
                nc.scalar.activation(Aeb, Aet, AFT.Copy)
                AeS = work.tile([32, 128], BF16, tag="AeS")
                nc.vector.tensor_scalar_mul(AeS, Aet, 2.0 * RHO)

                K_ps = pspool.tile([128, 128], F32, tag="ps")
                nc.tensor.matmul(K_ps, Aib, AiS, start=True, stop=False)
                nc.tensor.matmul(K_ps, Aeb, AeS, start=False, stop=True)
                # tmp = -rho*AtA - Q ; negK = tmp - cI ; X1 = c^2*tmp + (2c-c^2*a)I
                tmp = work.tile([128, 128], F32, tag="tmp")
                nc.vector.scalar_tensor_tensor(out=tmp, in0=K_ps, scalar=-1.0,
                                               in1=Qt, op0=ALU.mult,
                                               op1=ALU.subtract)
                negK = work.tile([128, 128], F32, tag="negK")
                nc.vector.scalar_tensor_tensor(out=negK, in0=tmp, scalar=1.0,
                                               in1=cIdent, op0=ALU.mult,
                                               op1=ALU.subtract)
                negKb = work.tile([128, 128], BF16, tag="negKb")
                nc.scalar.activation(negKb, negK, AFT.Copy)
                # fp32 X state; bf16 copies feed the matmuls (matches validated
                # numerics: only multiply operands are rounded)
                Xf = work.tile([128, 128], F32, tag="Xs")
                nc.vector.scalar_tensor_tensor(out=Xf, in0=tmp, scalar=C0 * C0,
                                               in1=xinitI, op0=ALU.mult,
                                               op1=ALU.add)
                for k in range(ns_loop - 2):
                    Xb = work.tile([128, 128], BF16, tag="X")
                    nc.scalar.activation(Xb, Xf, AFT.Copy)
                    G1_ps = pspool.tile([128, 128], F32, tag="ps")
                    nc.tensor.matmul(G1_ps, negKb, Xb, start=True, stop=True)
                    g1 = work.tile([128, 128], BF16, tag="g1")
                    nc.scalar.activation(g1, G1_ps, AFT.Copy)
                    X2_ps = pspool.tile([128, 128], F32, tag="ps")
                    nc.tensor.matmul(X2_ps, Xb, g1, start=True, stop=True)
                    Xn = work.tile([128, 128], F32, tag="Xs")
                    nc.vector.scalar_tensor_tensor(out=Xn, in0=Xf, scalar=2.0,
                                                   in1=X2_ps, op0=ALU.mult,
                                                   op1=ALU.add)
                    Xf = Xn
                # fp32 polish iteration: X8 = 2 Xf + Xf negK Xf.
                # Xf is not exactly symmetric (bf16 drift), and matmul uses
                # lhsT^T - so feed the explicit transpose of Xf as lhsT.
                XfT_ps = pspool.tile([128, 128], F32, tag="ps")
                nc.tensor.transpose(XfT_ps, Xf, ident)
                XfT = work.tile([128, 128], F32, tag="XfT")
                nc.vector.tensor_copy(XfT, XfT_ps)
                G1p = pspool.tile([128, 128], F32, tag="ps")
                nc.tensor.matmul(G1p, negK, Xf, start=True, stop=True)
                g1f = work.tile([128, 128], F32, tag="g1f")
                nc.scalar.activation(g1f, G1p, AFT.Copy)
                X2p = pspool.tile([128, 128], F32, tag="ps")
                nc.tensor.matmul(X2p, XfT, g1f, start=True, stop=False)
                nc.tensor.matmul(X2p, twoI, Xf, start=False, stop=True)
                X = work.tile([128, 128], F32, tag="X8")
                nc.vector.tensor_copy(X, X2p)

                # M = Kinv At' ; s_vec column into persistent Sbank
                Ms_ps = pspool.tile([128, MT], F32, tag="ps")
                nc.tensor.matmul(Ms_ps, X, AT, start=True, stop=True)
                nc.tensor.matmul(_col(Sbank, n), X, _col(nqv_all, n),
                                 start=True, stop=True, skip_group_check=True)
                Ms = work.tile([128, MT], F32, tag="Ms")
                nc.vector.tensor_copy(Ms, Ms_ps)

                # d columns (fp32) into persistent Dbank: d = M^T nqv = H nqv
                a_, q_ = n % 4, n // 4
                nc.tensor.matmul(_col(Dbank, n), Ms[:, 0:128], _col(nqv_all, n),
                                 start=True, stop=True, skip_group_check=True)
                nc.tensor.matmul(Dbank[0:32, n_el + n:n_el + n + 1],
                                 Ms[:, 128:160], _col(nqv_all, n),
                                 start=True, stop=True, skip_group_check=True)

                # H = At Kinv = Ms^T via PE transposes (single-pass)
                H_ps = pspool.tile([128, 256], F32, tag="ps")
                nc.tensor.transpose(H_ps[:, 0:128], Ms[:, 0:128], ident)
                nc.tensor.transpose(H_ps[0:32, 128:256], Ms[:, 128:160],
                                    ident)

                # G rows -> bf16 tiles (negated); bf16 inputs
                ATb = work.tile([128, MT], BF16, tag="ATb")
                nc.scalar.activation(ATb, AT, AFT.Copy)
                Msb = work.tile([128, MT], BF16, tag="Msb")
                nc.scalar.activation(Msb, Ms, AFT.Copy)
                Gr1_ps = pspool.tile([128, MT], F32, tag="ps")
                nc.tensor.matmul(Gr1_ps, ATb[:, 0:128], Msb, start=True, stop=True)
                Gr2_ps = pspool.tile([32, MT], F32, tag="ps")
                nc.tensor.matmul(Gr2_ps, ATb[:, 128:160], Msb, start=True, stop=True)
                nc.vector.tensor_scalar_mul(t1(n), Gr1_ps, -1.0)
                nc.vector.tensor_scalar_mul(
                    G2A_all[32 * a_:32 * a_ + 32, q_ * 128:(q_ + 1) * 128],
                    Gr2_ps[:, 0:128], -1.0)
                nc.vector.tensor_scalar_mul(g2e(n), Gr2_ps[:, 128:160], -1.0)

                Htile = work.tile([128, 256], F32, tag="H")
                nc.scalar.activation(Htile[:, 0:128], H_ps[:, 0:128], AFT.Copy)
                nc.scalar.activation(Htile[0:32, 128:256], H_ps[0:32, 128:256],
                                     AFT.Copy)
                nc.sync.dma_start(out=hsp_d[n, 0:128, :], in_=Htile[:, 0:128])
                nc.sync.dma_start(out=hsp_d[n, 128:160, :], in_=Htile[0:32, 128:256])

                if taps and n == 0:
                    nc.sync.dma_start(out=dbg_d[0, :, 0:128], in_=negK)
                    nc.sync.dma_start(out=dbg_d[1, :, 0:128], in_=X)
                    nc.sync.dma_start(out=dbg_d[2, :, 0:MT], in_=Ms)
                    nc.sync.dma_start(out=dbg_d[3, :, 0:128], in_=Xf)
                    nc.sync.dma_start(out=dbg_d[4, :, 0:128], in_=g1f)

            nc.vector.tensor_copy(S_all, Sbank)
            nc.vector.tensor_copy(D_all[:, 0:n_el], Dbank[:, 0:n_el])
            nc.vector.tensor_copy(D_all[0:32, n_el:2 * n_el],
                                  Dbank[0:32, n_el:2 * n_el])

            # ---------------- s1 init + C' prepass ----------------
            # top: psum = d_i - u_i (s1), then + g0_i (C')
            S1T = pspool.tile([128, n_el], F32, tag="ps")
            nc.tensor.matmul(S1T, negI, u_i, start=True, stop=False,
                             skip_group_check=True)
            nc.tensor.matmul(S1T, ident, D_all[:, 0:n_el], start=False, stop=False,
                             skip_group_check=True)
            nc.vector.tensor_copy(s_i[0], S1T)
            S1E = pspool.tile([32, n_el], F32, tag="ps")
            nc.tensor.matmul(S1E, negI[0:32, 0:32], u_e2, start=True, stop=False,
                             skip_group_check=True)
            nc.tensor.matmul(S1E, ident[0:32, 0:32],
                             D_all[0:32, n_el:2 * n_el], start=False, stop=False,
                             skip_group_check=True)
            nc.vector.tensor_copy(s_e[0][:, 0:n_el], S1E)
            nc.vector.tensor_scalar(out=s_e[0][:, n_el:2 * n_el], in0=S1E,
                                    scalar1=-1.0, scalar2=-EPS_,
                                    op0=ALU.mult, op1=ALU.add)

            # accumulate g0 terms (bf16 G x bf16 -rho*uC) into the same psums
            for n in range(n_el):
                a = n % 4
                last = n == n_el - 1
                nc.tensor.matmul(_col(S1T, n), t1(n)[:, 0:128], _col(nruC_top, n),
                                 start=False, stop=False, skip_group_check=True)
                nc.tensor.matmul(_col(S1E, n), t1(n)[:, 128:160],
                                 _col(nruC_top, n),
                                 start=False, stop=False, skip_group_check=True)
                nc.tensor.matmul(_col(S1E, n), g2e(n),
                                 nruC_bot[32 * a:32 * a + 32, n:n + 1],
                                 start=False, stop=last,
                                 skip_group_check=True, tile_position=(32 * a, 0))
            for q in range(Q):
                nc.tensor.matmul(S1T[:, 4 * q:4 * q + 4],
                                 G2A_all[:, q * 128:(q + 1) * 128],
                                 nruC_botD[:, 4 * q:4 * q + 4],
                                 start=False, stop=(q == Q - 1),
                                 skip_group_check=True)
            nc.vector.tensor_copy(Cp_i, S1T)
            nc.vector.tensor_copy(Cp_e[:, 0:n_el], S1E)
            nc.vector.tensor_scalar(out=Cp_e[:, n_el:2 * n_el], in0=S1E,
                                    scalar1=-1.0, scalar2=-EPS_,
                                    op0=ALU.mult, op1=ALU.add)
            if taps:
                nc.sync.dma_start(out=dbg_d[5, :, 0:n_el], in_=Cp_i)
                nc.sync.dma_start(out=dbg_d[6, :, 0:n_el], in_=s_i[0])

            # ---------------- phase B: ADMM loop ----------------
            def half_iter(src, dst):
                nc.scalar.activation(B_i[src], s_i[src], AFT.Abs, scale=RHO)
                nc.scalar.activation(B_e[src], s_e[src], AFT.Abs, scale=RHO)
                nc.scalar.activation(Bib[src], B_i[src], AFT.Copy)
                nc.vector.tensor_tensor(pbot[src][0:32, :], B_e[src][:, 0:n_el],
                                        B_e[src][:, n_el:2 * n_el], ALU.subtract)
                nc.vector.tensor_copy(pbot[src][32:64, :], pbot[src][0:32, :])
                nc.vector.tensor_copy(pbot[src][64:128, :], pbot[src][0:64, :])
                for k in range(4):
                    nc.vector.tensor_copy(
                        _strided_cols(pbotD[src], k, 4, Q,
                                      part=(32 * k, 32 * k + 32)),
                        _strided_cols(pbot[src], k, 4, Q,
                                      part=(32 * k, 32 * k + 32)))

                bankT = pspool.tile([128, n_el], F32, tag="ps")
                bankE = pspool.tile([32, 3 * n_el], F32, tag="ps")
                nc.tensor.matmul(bankT, ident, Cp_i, start=True, stop=False,
                                 skip_group_check=True)
                nc.tensor.matmul(bankT, hbrI, B_i[src], start=False, stop=False,
                                 skip_group_check=True)
                nc.tensor.matmul(bankT, halfI, s_i[src], start=False, stop=False,
                                 skip_group_check=True)
                nc.tensor.matmul(bankE[:, n_el:3 * n_el], ident[0:32, 0:32], Cp_e,
                                 start=True, stop=False, skip_group_check=True)
                nc.tensor.matmul(bankE[:, n_el:3 * n_el], hbrI[0:32, 0:32],
                                 B_e[src], start=False, stop=False,
                                 skip_group_check=True)
                nc.tensor.matmul(bankE[:, n_el:3 * n_el], halfI[0:32, 0:32],
                                 s_e[src], start=False, stop=False,
                                 skip_group_check=True)
                for q in range(Q):
                    nc.tensor.matmul(bankT[:, 4 * q:4 * q + 4],
                                     G2A_all[:, q * 128:(q + 1) * 128],
                                     pbotD[src][:, 4 * q:4 * q + 4],
                                     start=False, stop=False,
                                     skip_group_check=True)
                for n in range(n_el):
                    a = n % 4
                    last = n == n_el - 1
                    nc.tensor.matmul(_col(bankT, n), t1(n)[:, 0:128],
                                     _col(Bib[src], n), start=False, stop=last,
                                     skip_group_check=True)
                    nc.tensor.matmul(_col(bankE, n), t1(n)[:, 128:160],
                                     _col(Bib[src], n), start=True, stop=False,
                                     skip_group_check=True)
                    nc.tensor.matmul(_col(bankE, n), g2e(n),
                                     pbot[src][32 * a:32 * a + 32, n:n + 1],
                                     start=False, stop=last,
                                     skip_group_check=True,
                                     tile_position=(32 * a, 0))
                nc.vector.tensor_copy(s_i[dst], bankT)
                nc.scalar.activation(he_sb[src], bankE[:, 0:n_el], AFT.Copy)
                nc.vector.tensor_tensor(s_e[dst][:, 0:n_el],
                                        bankE[:, n_el:2 * n_el],
                                        he_sb[src], ALU.add)
                nc.vector.tensor_tensor(s_e[dst][:, n_el:2 * n_el],
                                        bankE[:, 2 * n_el:3 * n_el],
                                        he_sb[src], ALU.subtract)

            if n_body > 0:
                with tc.For_i(0, n_body, 1,
                              hint_engines=(mybir.EngineType.PE,)):
                    half_iter(0, 1)
                    half_iter(1, 0)

            # ---------------- final: x = M (rho uC - p~_99) + s_vec -------------
            nc.scalar.activation(B_i[0], s_i[0], AFT.Abs, scale=RHO)
            nc.scalar.activation(B_e[0], s_e[0], AFT.Abs, scale=RHO)
            nc.vector.tensor_tensor(f_bot, B_e[0][:, 0:n_el],
                                    B_e[0][:, n_el:2 * n_el], ALU.subtract)
            nc.vector.tensor_tensor(f_bot, ruC_bot, f_bot, ALU.subtract)
            nc.vector.tensor_tensor(f_top, ruC_top, B_i[0], ALU.subtract)

            xP = pspool.tile([128, n_el], F32, tag="ps")
            nc.tensor.matmul(xP, ident, S_all, start=True, stop=False,
                             skip_group_check=True)
            for n in range(n_el):
                Ht = hre.tile([128, 128], F32, tag="hret")
                nc.sync.dma_start(out=Ht, in_=hsp_d[n, 0:128, :])
                Hb = hre.tile([32, 128], F32, tag="hreb")
                nc.sync.dma_start(out=Hb, in_=hsp_d[n, 128:160, :])
                nc.tensor.matmul(_col(xP, n), Ht, _col(f_top, n),
                                 start=False, stop=False, skip_group_check=True)
                nc.tensor.matmul(_col(xP, n), Hb, _col(f_bot, n),
                                 start=False, stop=(n == n_el - 1),
                                 skip_group_check=True)
            nc.vector.tensor_copy(xo, xP)
            if taps:
                nc.sync.dma_start(out=dbg_d[7, :, 0:n_el], in_=s_i[0])
            xT = pspool.tile([n_el, 128], F32, tag="ps")
            nc.tensor.transpose(xT, xo, ident)
            nc.vector.tensor_copy(xout, xT)
            nc.sync.dma_start(out=out_d[0:n_el, :, 0], in_=xout)

    nc.compile()
    return nc


_NC_CACHE = {}


def _get_nc(taps=False):
    key = taps
    if key not in _NC_CACHE:
        _NC_CACHE[key] = build(taps=taps)
    return _NC_CACHE[key]


def run(inputs, taps=False, trace=False):
    nc = _get_nc(taps=taps)
    in_maps = []
    for c in range(NCORES):
        sl = slice(c * P, (c + 1) * P)
        in_maps.append({k: np.ascontiguousarray(np.asarray(v)[sl], dtype=np.float32)
                        for k, v in inputs.items()})
    res = run_bass_kernel_spmd(nc, in_maps, core_ids=list(range(NCORES)),
                               trace=trace)
    out = np.concatenate([res.results[c]["out"] for c in range(NCORES)], axis=0)
    return out, res


def kernel(**inputs):
    out, _ = run(inputs)
    return out
